# revision 27
# baseline (speedup 1.0000x reference)
"""Trainium2 Bass kernel for nn_AttentionBlock (GroupNorm + 1x1-conv QKV +
multi-head attention + 1x1-conv proj + residual).

Contract: kernel(**inputs) takes the FULL unsharded inputs (numpy) and
returns the FULL output.  Internally shards data-parallel over batch across
8 NeuronCores (2 samples per core).

v4 design (v1 baseline 548us -> v2 411us -> v4):
  - PE is the roofline engine (~200K cycles/sample).  The enemy is the HAM
    clock gate: any recurring PE micro-stall (waiting on exp or PSUM-slot
    reuse) re-throttles the PE to 1.2 GHz.  So the attention inner loop is
    paced such that exp always finishes well before its PSUM slot is needed:
    attention(sample0) is INTERLEAVED with qkv/vT(sample1) at head-pair
    granularity, and attention(sample1) with proj(sample0).  The extra PE
    work between slot reuses (~2x) gives the exp engines ample slack.
  - exp split per m-chunk: one fused [128,1024] DVE Schraudolph fast-exp
    (tensor_scalar f32->int16 round-to-nearest = bf16 bits of exp; max elem
    err ~3.3%, cancelled by softmax normalization) + one fused ScalarE
    exact Exp.  Z rides along as a ones-column in the v^T stationary.
  - 1/Z: Z rows copied out of PSUM on ScalarE, DMA-packed to [16,128],
    one DVE reciprocal_approx_fast per head-pair, DMA-flattened, gpsimd
    partition_broadcast, DVE multiply.
  - qkv bias+bf16 cast: single fused ScalarE Identity per output chunk.
    proj bias pre-folded into the residual source tiles (DVE, gn phase).
"""

import math
import os

import numpy as np

import concourse.bacc as bacc
import concourse.tile as tile
from concourse import mybir
from concourse.bass_utils import run_bass_kernel_spmd

F32 = mybir.dt.float32
I16 = mybir.dt.int16
ALU = mybir.AluOpType
ACT = mybir.ActivationFunctionType

N_CORES = 8
B, C, HH, WW = 16, 512, 32, 32
L = HH * WW            # 1024
BL = B // N_CORES      # batches per core = 2
NH = 8                 # heads
CH = C // NH           # head dim = 64
GROUPS = 32
GS = C // GROUPS       # channels per group = 16
EPS = 1e-5
SCALE2 = 1.0 / math.sqrt(CH)   # combined q*k scale, folded into exp
CT = C // 128          # channel tiles = 4
ST = L // 128          # s-chunks = 8
INV_N = 1.0 / (GS * L)         # group mean divisor
# Schraudolph fast-exp constants (bf16-bit-space, round-to-nearest):
#   i16 = round(p * EXP_A + EXP_B);  bf16_bits(i16) ~= exp(SCALE2 * p)
EXP_A = SCALE2 * math.log2(math.e) * 128.0   # 23.083120654223414
EXP_B = (127.0 - 0.044) * 128.0              # 16250.368

MM_DT = mybir.dt.bfloat16

LAST_RESULTS = None  # test harness can read exec_time_ns from here


def _build_program():
    nc = bacc.Bacc("TRN2", target_bir_lowering=False, debug=False,
                   num_devices=N_CORES)

    x_d = nc.dram_tensor("x", [BL, C, L], MM_DT, kind="ExternalInput").ap()
    out_d = nc.dram_tensor("out", [BL, C, L], F32, kind="ExternalOutput").ap()
    wqT_d = nc.dram_tensor("wqkvT", [C, 3 * C], MM_DT, kind="ExternalInput").ap()
    wpT_d = nc.dram_tensor("wprojT", [C, C], MM_DT, kind="ExternalInput").ap()
    nw_d = nc.dram_tensor("norm_w", [C], F32, kind="ExternalInput").ap()
    nb_d = nc.dram_tensor("norm_b", [C], F32, kind="ExternalInput").ap()
    qb_d = nc.dram_tensor("qkv_b", [3 * C], F32, kind="ExternalInput").ap()
    pb_d = nc.dram_tensor("proj_b", [C], F32, kind="ExternalInput").ap()
    sel_d = nc.dram_tensor("sel", [CT, 128, GROUPS], F32, kind="ExternalInput").ap()
    fan_d = nc.dram_tensor("fan", [CT, GROUPS, 128], F32, kind="ExternalInput").ap()
    ones_d = nc.dram_tensor("ones", [1, 512], MM_DT, kind="ExternalInput").ap()

    with tile.TileContext(nc) as tc:
        with (
            tc.tile_pool(name="wgt", bufs=1) as wgt,
            tc.tile_pool(name="xs", bufs=2 * CT) as xs_p,
            tc.tile_pool(name="xn", bufs=CT + 1) as xn_p,
            tc.tile_pool(name="qk", bufs=4 * CT) as qk_p,
            tc.tile_pool(name="vt", bufs=2 * ST) as vt_p,
            tc.tile_pool(name="ew", bufs=2 * ST) as ew_p,
            tc.tile_pool(name="apool", bufs=2 * CT) as a_p,
            tc.tile_pool(name="zz", bufs=2) as z_p,
            tc.tile_pool(name="zb", bufs=2) as zb_p,
            tc.tile_pool(name="stg", bufs=4) as stg_p,
            tc.tile_pool(name="outs", bufs=2) as out_p,
            tc.tile_pool(name="tiny", bufs=16) as tiny,
            tc.tile_pool(name="scr", bufs=4) as scr_p,
            tc.tile_pool(name="pmm", bufs=3, space="PSUM") as pmm,   # 3x 2-bank
            tc.tile_pool(name="pa", bufs=2, space="PSUM") as pa_p,   # 2x 1-bank
        ):
            # ---------------- constants / weights ----------------
            # Allocation order fixed (SBUF layout); DMA issue order prioritizes
            # the GroupNorm critical path: x(0)+gn constants on the sync queue,
            # bulk weights + x(1) in parallel on the gpsimd queue.
            wq, wp, sel, fan, nw, nb = [], [], [], [], [], []
            for i in range(CT):
                wq.append(wgt.tile([128, 3 * C], MM_DT, tag=f"wq{i}", name=f"wq{i}"))
                wp.append(wgt.tile([128, C], MM_DT, tag=f"wp{i}", name=f"wp{i}"))
                sel.append(wgt.tile([128, GROUPS], F32, tag=f"sel{i}", name=f"sel{i}"))
                fan.append(wgt.tile([GROUPS, 128], F32, tag=f"fan{i}", name=f"fan{i}"))
                nw.append(wgt.tile([128, 1], F32, tag=f"nw{i}", name=f"nw{i}"))
                nb.append(wgt.tile([128, 1], F32, tag=f"nb{i}", name=f"nb{i}"))
            ones_t = wgt.tile([1, 512], MM_DT, tag="ones")
            qbv_t = wgt.tile([1, C], MM_DT, tag="qbv")
            qbv_f = wgt.tile([1, C], F32, tag="qbvf")
            qb_qk = [wgt.tile([128, 1], F32, tag=f"qb{j}", name=f"qb{j}")
                     for j in range(2 * CT)]
            pb_col = [wgt.tile([128, 1], F32, tag=f"pbc{j}", name=f"pbc{j}")
                      for j in range(CT)]
            eps_t = wgt.tile([GROUPS, 1], F32, tag="eps")
            xs = {}
            for b in range(BL):
                for i in range(CT):
                    xs[(b, i)] = xs_p.tile([128, L], MM_DT, tag="xs",
                                           name=f"xs{b}_{i}")

            # warm-up stationary carved out of the outs pool (no DMA needed)
            warm_f = out_p.tile([128, L], F32, tag="o", name="warm_f")
            warm_t = warm_f[:].bitcast(MM_DT)
            nc.vector.memset(warm_t[:, 0:512], 1.0)
            nc.vector.memset(eps_t[:], EPS)

            # DMA issue order: gn-critical first
            for i in range(CT):
                nc.sync.dma_start(xs[(0, i)][:], x_d[0, 128 * i:128 * (i + 1), :])
            for i in range(CT):
                nc.sync.dma_start(sel[i][:], sel_d[i, :, :])
                nc.sync.dma_start(fan[i][:], fan_d[i, :, :])
                nc.sync.dma_start(nw[i][:], nw_d[128 * i:128 * (i + 1)].rearrange("(p a) -> p a", a=1))
                nc.sync.dma_start(nb[i][:], nb_d[128 * i:128 * (i + 1)].rearrange("(p a) -> p a", a=1))
            for i in range(CT):
                nc.gpsimd.dma_start(wq[i][:], wqT_d[128 * i:128 * (i + 1), :])
            for i in range(CT):
                nc.gpsimd.dma_start(xs[(1, i)][:], x_d[1, 128 * i:128 * (i + 1), :])
            for i in range(CT):
                nc.gpsimd.dma_start(wp[i][:], wpT_d[128 * i:128 * (i + 1), :])
            nc.gpsimd.dma_start(ones_t[:], ones_d[:, :])
            nc.gpsimd.dma_start(qbv_f[:], qb_d[2 * C:3 * C].rearrange("(a b) -> a b", a=1))
            nc.vector.tensor_copy(qbv_t[:], qbv_f[:])
            for j in range(2 * CT):
                nc.gpsimd.dma_start(qb_qk[j][:], qb_d[128 * j:128 * (j + 1)].rearrange("(p a) -> p a", a=1))
            for j in range(CT):
                nc.gpsimd.dma_start(pb_col[j][:], pb_d[128 * j:128 * (j + 1)].rearrange("(p a) -> p a", a=1))

            xn = {}
            qk = {}
            vt = {}
            a_tiles = {}

            gn_stats = {}

            def gn_pre(b):
                stats = []
                for i in range(CT):
                    bns = scr_p.tile([128, 2, 6], F32, tag="bns", name=f"bns{b}_{i}")
                    xv = xs[(b, i)][:].rearrange("p (s f) -> p s f", f=512)
                    for sgi in range(2):
                        nc.vector.bn_stats(bns[:, sgi, :], xv[:, sgi, :])
                    mv = tiny.tile([128, 2], F32, tag="mv", name=f"mv{b}_{i}")
                    nc.vector.bn_aggr(mv[:], bns[:])
                    st = tiny.tile([128, 2], F32, tag="stats", name=f"st{b}_{i}")
                    nc.vector.tensor_scalar_mul(st[:, 0:1], mv[:, 0:1], float(L))
                    m2 = tiny.tile([128, 2], F32, tag="m2", name=f"m2{b}_{i}")
                    nc.vector.tensor_mul(m2[:, 0:1], mv[:, 0:1], mv[:, 0:1])
                    nc.vector.tensor_add(m2[:, 1:2], mv[:, 1:2], m2[:, 0:1])
                    nc.vector.tensor_scalar_mul(st[:, 1:2], m2[:, 1:2], float(L))
                    stats.append(st)
                gn_stats[b] = stats

            def gn_post(b, pad=False):
                stats = gn_stats[b]
                pg = pmm.tile([128, 1024], F32, tag="mm", name=f"pg{b}")
                for i in range(CT):
                    nc.tensor.matmul(pg[0:GROUPS, 0:2], sel[i][:, :], stats[i][:, :],
                                     start=(i == 0), stop=(i == CT - 1))
                if pad:
                    warm(8)
                gq = tiny.tile([GROUPS, 8], F32, tag="gq", name=f"gq{b}")
                nc.vector.tensor_scalar_mul(gq[:, 0:1], pg[0:GROUPS, 0:1], INV_N)
                nc.vector.tensor_scalar_mul(gq[:, 1:2], pg[0:GROUPS, 1:2], INV_N)
                nc.vector.tensor_mul(gq[:, 2:3], gq[:, 0:1], gq[:, 0:1])
                nc.vector.tensor_sub(gq[:, 3:4], gq[:, 1:2], gq[:, 2:3])
                # rstd = rsqrt(var+eps) on DVE: magic-constant seed + 2 Newton
                # steps (keeps ScalarE's activation table pinned to Exp/Copy)
                vr = tiny.tile([GROUPS, 6], F32, tag="vr", name=f"vr{b}")
                vi = vr[:].bitcast(mybir.dt.int32)
                nc.vector.tensor_scalar_add(vr[:, 0:1], gq[:, 3:4], EPS)
                nc.vector.tensor_scalar(vi[:, 1:2], vi[:, 0:1], 1, None,
                                        op0=ALU.logical_shift_right)
                nc.vector.tensor_scalar(vi[:, 2:3], vi[:, 1:2], -1, 0x5f3759df,
                                        op0=ALU.mult, op1=ALU.add)
                for it in (3, 4):
                    nc.vector.tensor_mul(vr[:, 5:6], vr[:, 0:1],
                                         vr[:, it - 1:it])
                    nc.vector.tensor_mul(vr[:, 5:6], vr[:, 5:6],
                                         vr[:, it - 1:it])
                    nc.vector.tensor_scalar(vr[:, 5:6], vr[:, 5:6], -0.5, 1.5,
                                            op0=ALU.mult, op1=ALU.add)
                    nc.vector.tensor_mul(vr[:, it:it + 1], vr[:, it - 1:it],
                                         vr[:, 5:6])
                nc.vector.tensor_copy(gq[:, 5:6], vr[:, 4:5])
                nc.vector.tensor_mul(gq[:, 6:7], gq[:, 0:1], gq[:, 5:6])
                for i in range(CT):
                    pf = pmm.tile([128, 1024], F32, tag="mm", name=f"pf{b}_{i}")
                    nc.tensor.matmul(pf[0:128, 0:2], fan[i][:, :], gq[:, 5:7],
                                     start=True, stop=True)
                    scb = tiny.tile([128, 3], F32, tag="scb", name=f"scb{b}_{i}")
                    nc.vector.tensor_mul(scb[:, 0:1], pf[0:128, 0:1], nw[i][:])
                    nc.vector.tensor_mul(scb[:, 1:2], pf[0:128, 1:2], nw[i][:])
                    nc.vector.tensor_sub(scb[:, 2:3], nb[i][:], scb[:, 1:2])
                    t = xn_p.tile([128, L], MM_DT, tag="xn", name=f"xn{b}_{i}")
                    nc.vector.tensor_scalar(t[:], xs[(b, i)][:], scb[:, 0:1],
                                            scb[:, 2:3], op0=ALU.mult, op1=ALU.add)
                    xn[(b, i)] = t
                    # fold proj bias into the residual source (in place)
                    nc.vector.tensor_scalar_add(xs[(b, i)][:], xs[(b, i)][:],
                                                pb_col[i][:])
                if pad:
                    warm(10)

            def qkv_j(b, j):
                t = qk_p.tile([128, L], MM_DT, tag="qk", name=f"qk{b}_{j}")
                pq = pmm.tile([128, 1024], F32, tag="mm", name=f"pq{b}_{j}")
                for i in range(CT):
                    for n in range(2):
                        nsl = slice(512 * n, 512 * (n + 1))
                        nc.tensor.matmul(pq[:, nsl],
                                         wq[i][:, 128 * j:128 * (j + 1)],
                                         xn[(b, i)][:, nsl],
                                         start=(i == 0), stop=(i == CT - 1))
                nc.scalar.activation(t[:, :], pq[:, :], ACT.Identity,
                                     bias=qb_qk[j][:])
                qk[(b, j)] = t

            def vt_m(b, m):
                pvf = pmm.tile([128, 1024], F32, tag="mm", name=f"pv{b}_{m}")
                pv = pvf[:, 0:512]
                for i in range(CT):
                    nc.tensor.matmul(pv,
                                     xn[(b, i)][:, 128 * m:128 * (m + 1)],
                                     wq[i][:, 2 * C:3 * C],
                                     start=(i == 0), stop=False)
                nc.tensor.matmul(pv, ones_t[0:1, 0:128], qbv_t[0:1, :],
                                 start=False, stop=True)
                t = vt_p.tile([128, NH * (CH + 1)], MM_DT, tag="vt",
                              name=f"vt{b}_{m}")
                dst = t[:].rearrange("p (h x) -> p h x", x=CH + 1)
                nc.vector.tensor_copy(dst[:, :, 0:CH],
                                      pv.rearrange("p (h x) -> p h x", x=CH))
                nc.vector.memset(dst[:, :, CH:CH + 1], 1.0)
                vt[(b, m)] = t

            deferred = []   # normalize-emission units of the previous hp

            def defer_step(force=False):
                if deferred:
                    deferred.pop(0)()

            def attn_hp(b, hp, fctx=None):

                def fill_step(force=False):
                    if fctx is None or not fctx["fills"]:
                        return
                    if force:
                        fctx["fills"].pop(0)()
                        return
                    k = fctx["step"]
                    fctx["step"] += 1
                    if (k + 1) * fctx["U"] // fctx["S"] > k * fctx["U"] // fctx["S"]:
                        fctx["fills"].pop(0)()

                q_t = qk[(b, hp)]
                k_t = qk[(b, CT + hp)]
                ews = {}
                # ---- pass 1: scores + exp (full t) + av over t-half 0 ----
                pa_h = [pa_p.tile([CH + 1, 512], F32, tag="pa",
                                  name=f"pa0_{b}_{hp}_{hh}") for hh in range(2)]
                for m in range(ST):
                    defer_step()
                    msl = slice(128 * m, 128 * (m + 1))
                    pw = []
                    for hh, (plo, phi, tp) in enumerate(
                            ((0, CH, (0, 0)), (CH, 128, (CH, 0)))):
                        p = pmm.tile([128, 1024], F32, tag="mm",
                                     name=f"ps{b}_{hp}_{m}_{hh}")
                        for n in range(2):
                            nsl = slice(512 * n, 512 * (n + 1))
                            nc.tensor.matmul(p[:, nsl], k_t[plo:phi, msl],
                                             q_t[plo:phi, nsl],
                                             start=True, stop=True,
                                             tile_position=tp)
                        pw.append((hh, p))
                    for hh, p in pw:
                        if (hh + m) % 2 == 0:
                            e = ew_p.tile([128, L], I16, tag="ewi")
                            nc.vector.tensor_scalar(e[:], p[:, :], EXP_A, EXP_B,
                                                    op0=ALU.mult, op1=ALU.add)
                            ews[(m, hh)] = e.bitcast(MM_DT)
                        else:
                            e = ew_p.tile([128, L], MM_DT, tag="ewa")
                            nc.scalar.activation(e[:], p[:, :], ACT.Exp,
                                                 scale=SCALE2)
                            ews[(m, hh)] = e[:]
                    if m > 0:
                        for hh in range(2):
                            lhs = vt[(b, m - 1)][:, (2 * hp + hh) * (CH + 1):
                                                 (2 * hp + hh + 1) * (CH + 1)]
                            nc.tensor.matmul(pa_h[hh][:, :], lhs,
                                             ews[(m - 1, hh)][:, 0:512],
                                             start=(m - 1 == 0), stop=False)
                    fill_step()
                for hh in range(2):
                    lhs = vt[(b, ST - 1)][:, (2 * hp + hh) * (CH + 1):
                                          (2 * hp + hh + 1) * (CH + 1)]
                    nc.tensor.matmul(pa_h[hh][:, :], lhs,
                                     ews[(ST - 1, hh)][:, 0:512],
                                     start=False, stop=True)
                # stage t-half 0 out of PSUM (frees the banks for pass 2)
                stg = {}
                for hh in range(2):
                    s_t = stg_p.tile([CH + 1, 512], F32, tag="stg",
                                     name=f"stg{b}_{hp}_{hh}_0")
                    nc.vector.tensor_copy(s_t[:], pa_h[hh][:, :])
                    stg[(hh, 0)] = s_t
                zp0 = z_p.tile([8, 128], F32, tag="zp", name=f"zp0_{b}_{hp}")
                for hh in range(2):
                    nc.sync.dma_start(zp0[4 * hh:4 * hh + 4, :],
                                      stg[(hh, 0)][CH:CH + 1, :])
                zrp0 = z_p.tile([8, 128], F32, tag="zrp", name=f"zrp0_{b}_{hp}")
                nc.vector.reciprocal_approx_fast(zrp0[:], zp0[:])
                fill_step(force=True)
                # ---- pass 2: av over t-half 1 (ew tiles already in SBUF) ----
                pa_h2 = [pa_p.tile([CH + 1, 512], F32, tag="pa",
                                   name=f"pa1_{b}_{hp}_{hh}") for hh in range(2)]
                for m in range(ST):
                    for hh in range(2):
                        lhs = vt[(b, m)][:, (2 * hp + hh) * (CH + 1):
                                         (2 * hp + hh + 1) * (CH + 1)]
                        nc.tensor.matmul(pa_h2[hh][:, :], lhs,
                                         ews[(m, hh)][:, 512:1024],
                                         start=(m == 0), stop=(m == ST - 1))

                # ---- normalize: DEFERRED into the next hp's emission so the
                # stage/recip chain never blocks the exp stream in the FIFOs
                a_t = a_p.tile([128, L], MM_DT, tag="a", name=f"a{b}_{hp}")

                def u1():
                    for hh in range(2):
                        s_t = stg_p.tile([CH + 1, 512], F32, tag="stg",
                                         name=f"stg{b}_{hp}_{hh}_1")
                        nc.vector.tensor_copy(s_t[:], pa_h2[hh][:, :])
                        stg[(hh, 1)] = s_t

                zc = {0: zrp0}

                def u2():
                    zp1 = z_p.tile([8, 128], F32, tag="zp",
                                   name=f"zp1_{b}_{hp}")
                    for hh in range(2):
                        nc.sync.dma_start(zp1[4 * hh:4 * hh + 4, :],
                                          stg[(hh, 1)][CH:CH + 1, :])
                    zrp1 = z_p.tile([8, 128], F32, tag="zrp",
                                    name=f"zrp1_{b}_{hp}")
                    nc.vector.reciprocal_approx_fast(zrp1[:], zp1[:])
                    zc[1] = zrp1

                def mk_u34(hh):
                    def u():
                        for n in range(2):
                            rzf = z_p.tile([1, 512], F32, tag="rzf",
                                           name=f"rzf{b}_{hp}_{hh}_{n}")
                            nc.sync.dma_start(rzf[0:1, :],
                                              zc[n][4 * hh:4 * hh + 4, :])
                            rzb = zb_p.tile([CH, 512], F32, tag="zb")
                            nc.gpsimd.partition_broadcast(rzb[:], rzf[:])
                            rows = slice(CH * hh, CH * (hh + 1))
                            nsl = slice(512 * n, 512 * (n + 1))
                            nc.vector.tensor_mul(a_t[rows, nsl],
                                                 stg[(hh, n)][0:CH, :],
                                                 rzb[:, :])
                    return u

                deferred.extend([u1, u2, mk_u34(0), mk_u34(1)])
                a_tiles[(b, hp)] = a_t

            def proj_j(b, j):
                o_t = out_p.tile([128, L], F32, tag="o", name=f"o{b}_{j}")
                pp = pmm.tile([128, 1024], F32, tag="mm", name=f"pp{b}_{j}")
                for i in range(CT):
                    for n in range(2):
                        nsl = slice(512 * n, 512 * (n + 1))
                        nc.tensor.matmul(pp[:, nsl],
                                         wp[i][:, 128 * j:128 * (j + 1)],
                                         a_tiles[(b, i)][:, nsl],
                                         start=(i == 0), stop=(i == CT - 1))
                nc.vector.tensor_add(o_t[:, :], pp[:, :], xs[(b, j)][:, :])
                nc.sync.dma_start(out_d[b, 128 * j:128 * (j + 1), :], o_t[:])

            # ---------------- schedule ----------------
            # PE warm-up: real-shaped dummy matmuls while DMA/stats run, so
            # the HAM un-throttles before qkv(0) begins
            def warm(n):
                wu = pmm.tile([128, 1024], F32, tag="mm",
                              name=f"warm{nc.next_id()}")
                for _ in range(n):
                    nc.tensor.matmul(wu[:, 0:512], warm_t[:, 0:128],
                                     warm_t[:, 0:512], start=True, stop=True)

            warm(56)
            gn_pre(0)
            gn_post(0, pad=True)
            for j in range(2 * CT):
                qkv_j(0, j)
                if j == 0:
                    gn_pre(1)      # sample-1 stats on DVE under qkv(0) PE work
            for m in range(ST):
                vt_m(0, m)
            gn_post(1)
            # attention(0) interleaved per-m with qkv/vT of sample 1
            fills1 = ([(lambda jj: (lambda: qkv_j(1, jj)))(j) for j in range(2 * CT)]
                      + [(lambda mm_: (lambda: vt_m(1, mm_)))(m) for m in range(ST)])
            fctx1 = {"fills": fills1, "step": 0, "U": len(fills1), "S": 32}
            for hp in range(NH // 2):
                attn_hp(0, hp, fctx1)
            while fctx1["fills"]:
                fctx1["fills"].pop(0)()
            # attention(1) interleaved per-m with proj of sample 0
            fills2 = [(lambda jj: (lambda: proj_j(0, jj)))(j) for j in range(CT)]
            fctx2 = {"fills": fills2, "step": 0, "U": len(fills2), "S": 34}
            for hp in range(NH // 2):
                attn_hp(1, hp, fctx2)
            while deferred:
                deferred.pop(0)()
            while fctx2["fills"]:
                fctx2["fills"].pop(0)()
            for j in range(CT):
                proj_j(1, j)

    nc.compile()
    return nc


_prog_cache = {}


def _get_program():
    if "p" not in _prog_cache:
        _prog_cache["p"] = _build_program()
    return _prog_cache["p"]


def _host_constants():
    sel = np.zeros((CT, 128, GROUPS), dtype=np.float32)
    fan = np.zeros((CT, GROUPS, 128), dtype=np.float32)
    for i in range(CT):
        for p in range(128):
            g = (128 * i + p) // GS
            sel[i, p, g] = 1.0
            fan[i, g, p] = 1.0
    ones = np.ones((1, 512), dtype=mybir.dt.np(MM_DT))
    return sel, fan, ones


def kernel(x, norm_w, norm_b, qkv_w, qkv_b, proj_w, proj_b):
    global LAST_RESULTS
    x = np.ascontiguousarray(np.asarray(x, dtype=np.float32).astype(mybir.dt.np(MM_DT)))
    np_mm = mybir.dt.np(MM_DT)
    wqkvT = np.ascontiguousarray(np.asarray(qkv_w, dtype=np.float32).T.astype(np_mm))
    wprojT = np.ascontiguousarray(np.asarray(proj_w, dtype=np.float32).T.astype(np_mm))
    sel, fan, ones = _host_constants()

    xr = x.reshape(B, C, L)
    nc = _get_program()

    common = {
        "wqkvT": wqkvT,
        "wprojT": wprojT,
        "norm_w": np.ascontiguousarray(norm_w, dtype=np.float32),
        "norm_b": np.ascontiguousarray(norm_b, dtype=np.float32),
        "qkv_b": np.ascontiguousarray(qkv_b, dtype=np.float32),
        "proj_b": np.ascontiguousarray(proj_b, dtype=np.float32),
        "sel": sel,
        "fan": fan,
        "ones": ones,
    }
    in_maps = []
    for c in range(N_CORES):
        m = dict(common)
        m["x"] = np.ascontiguousarray(xr[BL * c:BL * (c + 1)])
        in_maps.append(m)

    trace = os.environ.get("KERNEL_TRACE", "0") == "1"
    kwargs = {}
    if trace:
        kwargs = dict(trace=True, trace_cores=[0])
    res = run_bass_kernel_spmd(nc, in_maps, core_ids=list(range(N_CORES)),
                               **kwargs)
    LAST_RESULTS = res
    out = np.concatenate([res.results[c]["out"] for c in range(N_CORES)], axis=0)
    return out.reshape(B, C, HH, WW)


# revision 28
# speedup vs baseline: 1.1849x; 1.1849x over previous
"""Trainium2 Bass kernel for nn_AttentionBlock (GroupNorm + 1x1-conv QKV +
multi-head attention + 1x1-conv proj + residual).

Contract: kernel(**inputs) takes the FULL unsharded inputs (numpy) and
returns the FULL output.  Internally shards data-parallel over batch across
8 NeuronCores (2 samples per core).

v4 design (v1 baseline 548us -> v2 411us -> v4):
  - PE is the roofline engine (~200K cycles/sample).  The enemy is the HAM
    clock gate: any recurring PE micro-stall (waiting on exp or PSUM-slot
    reuse) re-throttles the PE to 1.2 GHz.  So the attention inner loop is
    paced such that exp always finishes well before its PSUM slot is needed:
    attention(sample0) is INTERLEAVED with qkv/vT(sample1) at head-pair
    granularity, and attention(sample1) with proj(sample0).  The extra PE
    work between slot reuses (~2x) gives the exp engines ample slack.
  - exp split per m-chunk: one fused [128,1024] DVE Schraudolph fast-exp
    (tensor_scalar f32->int16 round-to-nearest = bf16 bits of exp; max elem
    err ~3.3%, cancelled by softmax normalization) + one fused ScalarE
    exact Exp.  Z rides along as a ones-column in the v^T stationary.
  - 1/Z: Z rows copied out of PSUM on ScalarE, DMA-packed to [16,128],
    one DVE reciprocal_approx_fast per head-pair, DMA-flattened, gpsimd
    partition_broadcast, DVE multiply.
  - qkv bias+bf16 cast: single fused ScalarE Identity per output chunk.
    proj bias pre-folded into the residual source tiles (DVE, gn phase).
"""

import math
import os

import numpy as np

import concourse.bacc as bacc
import concourse.tile as tile
from concourse import mybir
from concourse.bass_utils import run_bass_kernel_spmd

F32 = mybir.dt.float32
I16 = mybir.dt.int16
ALU = mybir.AluOpType
ACT = mybir.ActivationFunctionType

N_CORES = 8
B, C, HH, WW = 16, 512, 32, 32
L = HH * WW            # 1024
BL = B // N_CORES      # batches per core = 2
NH = 8                 # heads
CH = C // NH           # head dim = 64
GROUPS = 32
GS = C // GROUPS       # channels per group = 16
EPS = 1e-5
SCALE2 = 1.0 / math.sqrt(CH)   # combined q*k scale, folded into exp
CT = C // 128          # channel tiles = 4
ST = L // 128          # s-chunks = 8
INV_N = 1.0 / (GS * L)         # group mean divisor
# Schraudolph fast-exp constants (bf16-bit-space, round-to-nearest):
#   i16 = round(p * EXP_A + EXP_B);  bf16_bits(i16) ~= exp(SCALE2 * p)
EXP_A = SCALE2 * math.log2(math.e) * 128.0   # 23.083120654223414
EXP_B = (127.0 - 0.044) * 128.0              # 16250.368

MM_DT = mybir.dt.bfloat16

LAST_RESULTS = None  # test harness can read exec_time_ns from here


def _build_program():
    nc = bacc.Bacc("TRN2", target_bir_lowering=False, debug=False,
                   num_devices=N_CORES)

    x_d = nc.dram_tensor("x", [BL, C, L], MM_DT, kind="ExternalInput").ap()
    out_d = nc.dram_tensor("out", [BL, C, L], F32, kind="ExternalOutput").ap()
    wqT_d = nc.dram_tensor("wqkvT", [C, 3 * C], MM_DT, kind="ExternalInput").ap()
    wpT_d = nc.dram_tensor("wprojT", [C, C], MM_DT, kind="ExternalInput").ap()
    nw_d = nc.dram_tensor("norm_w", [C], F32, kind="ExternalInput").ap()
    nb_d = nc.dram_tensor("norm_b", [C], F32, kind="ExternalInput").ap()
    qb_d = nc.dram_tensor("qkv_b", [3 * C], F32, kind="ExternalInput").ap()
    pb_d = nc.dram_tensor("proj_b", [C], F32, kind="ExternalInput").ap()
    sel_d = nc.dram_tensor("sel", [CT, 128, GROUPS], F32, kind="ExternalInput").ap()
    fan_d = nc.dram_tensor("fan", [CT, GROUPS, 128], F32, kind="ExternalInput").ap()
    ones_d = nc.dram_tensor("ones", [1, 512], MM_DT, kind="ExternalInput").ap()

    with tile.TileContext(nc) as tc:
        with (
            tc.tile_pool(name="wgt", bufs=1) as wgt,
            tc.tile_pool(name="xs", bufs=2 * CT) as xs_p,
            tc.tile_pool(name="xn", bufs=CT + 1) as xn_p,
            tc.tile_pool(name="qk", bufs=4 * CT) as qk_p,
            tc.tile_pool(name="vt", bufs=2 * ST) as vt_p,
            tc.tile_pool(name="ew", bufs=2 * ST) as ew_p,
            tc.tile_pool(name="apool", bufs=2 * CT) as a_p,
            tc.tile_pool(name="zz", bufs=2) as z_p,
            tc.tile_pool(name="zb", bufs=2) as zb_p,
            tc.tile_pool(name="stg", bufs=4) as stg_p,
            tc.tile_pool(name="outs", bufs=2) as out_p,
            tc.tile_pool(name="tiny", bufs=16) as tiny,
            tc.tile_pool(name="scr", bufs=4) as scr_p,
            tc.tile_pool(name="pmm", bufs=3, space="PSUM") as pmm,   # 3x 2-bank
            tc.tile_pool(name="pa", bufs=2, space="PSUM") as pa_p,   # 2x 1-bank
        ):
            # ---------------- constants / weights ----------------
            # Allocation order fixed (SBUF layout); DMA issue order prioritizes
            # the GroupNorm critical path: x(0)+gn constants on the sync queue,
            # bulk weights + x(1) in parallel on the gpsimd queue.
            wq, wp, sel, fan, nw, nb = [], [], [], [], [], []
            for i in range(CT):
                wq.append(wgt.tile([128, 3 * C], MM_DT, tag=f"wq{i}", name=f"wq{i}"))
                wp.append(wgt.tile([128, C], MM_DT, tag=f"wp{i}", name=f"wp{i}"))
                sel.append(wgt.tile([128, GROUPS], F32, tag=f"sel{i}", name=f"sel{i}"))
                fan.append(wgt.tile([GROUPS, 128], F32, tag=f"fan{i}", name=f"fan{i}"))
                nw.append(wgt.tile([128, 1], F32, tag=f"nw{i}", name=f"nw{i}"))
                nb.append(wgt.tile([128, 1], F32, tag=f"nb{i}", name=f"nb{i}"))
            ones_t = wgt.tile([1, 512], MM_DT, tag="ones")
            qbv_t = wgt.tile([1, C], MM_DT, tag="qbv")
            qbv_f = wgt.tile([1, C], F32, tag="qbvf")
            qb_qk = [wgt.tile([128, 1], F32, tag=f"qb{j}", name=f"qb{j}")
                     for j in range(2 * CT)]
            pb_col = [wgt.tile([128, 1], F32, tag=f"pbc{j}", name=f"pbc{j}")
                      for j in range(CT)]
            eps_t = wgt.tile([GROUPS, 1], F32, tag="eps")
            xs = {}
            for b in range(BL):
                for i in range(CT):
                    xs[(b, i)] = xs_p.tile([128, L], MM_DT, tag="xs",
                                           name=f"xs{b}_{i}")

            # warm-up stationary carved out of the outs pool (no DMA needed)
            warm_f = out_p.tile([128, L], F32, tag="o", name="warm_f")
            warm_t = warm_f[:].bitcast(MM_DT)
            nc.vector.memset(warm_t[:, 0:512], 1.0)
            nc.vector.memset(eps_t[:], EPS)

            # DMA issue order: gn-critical first
            for i in range(CT):
                nc.sync.dma_start(xs[(0, i)][:], x_d[0, 128 * i:128 * (i + 1), :])
            for i in range(CT):
                nc.sync.dma_start(sel[i][:], sel_d[i, :, :])
                nc.sync.dma_start(fan[i][:], fan_d[i, :, :])
                nc.sync.dma_start(nw[i][:], nw_d[128 * i:128 * (i + 1)].rearrange("(p a) -> p a", a=1))
                nc.sync.dma_start(nb[i][:], nb_d[128 * i:128 * (i + 1)].rearrange("(p a) -> p a", a=1))
            for i in range(CT):
                nc.gpsimd.dma_start(wq[i][:], wqT_d[128 * i:128 * (i + 1), :])
            for i in range(CT):
                nc.gpsimd.dma_start(xs[(1, i)][:], x_d[1, 128 * i:128 * (i + 1), :])
            for i in range(CT):
                nc.gpsimd.dma_start(wp[i][:], wpT_d[128 * i:128 * (i + 1), :])
            nc.gpsimd.dma_start(ones_t[:], ones_d[:, :])
            nc.gpsimd.dma_start(qbv_f[:], qb_d[2 * C:3 * C].rearrange("(a b) -> a b", a=1))
            nc.vector.tensor_copy(qbv_t[:], qbv_f[:])
            for j in range(2 * CT):
                nc.gpsimd.dma_start(qb_qk[j][:], qb_d[128 * j:128 * (j + 1)].rearrange("(p a) -> p a", a=1))
            for j in range(CT):
                nc.gpsimd.dma_start(pb_col[j][:], pb_d[128 * j:128 * (j + 1)].rearrange("(p a) -> p a", a=1))

            xn = {}
            qk = {}
            vt = {}
            a_tiles = {}

            gn_stats = {}

            def gn_pre(b):
                stats = []
                for i in range(CT):
                    bns = scr_p.tile([128, 2, 6], F32, tag="bns", name=f"bns{b}_{i}")
                    xv = xs[(b, i)][:].rearrange("p (s f) -> p s f", f=512)
                    for sgi in range(2):
                        nc.vector.bn_stats(bns[:, sgi, :], xv[:, sgi, :])
                    mv = tiny.tile([128, 2], F32, tag="mv", name=f"mv{b}_{i}")
                    nc.vector.bn_aggr(mv[:], bns[:])
                    st = tiny.tile([128, 2], F32, tag="stats", name=f"st{b}_{i}")
                    nc.vector.tensor_scalar_mul(st[:, 0:1], mv[:, 0:1], float(L))
                    m2 = tiny.tile([128, 2], F32, tag="m2", name=f"m2{b}_{i}")
                    nc.vector.tensor_mul(m2[:, 0:1], mv[:, 0:1], mv[:, 0:1])
                    nc.vector.tensor_add(m2[:, 1:2], mv[:, 1:2], m2[:, 0:1])
                    nc.vector.tensor_scalar_mul(st[:, 1:2], m2[:, 1:2], float(L))
                    stats.append(st)
                gn_stats[b] = stats

            def gn_post(b):
                stats = gn_stats[b]
                pg = pmm.tile([128, 1024], F32, tag="mm", name=f"pg{b}")
                for i in range(CT):
                    nc.tensor.matmul(pg[0:GROUPS, 0:2], sel[i][:, :], stats[i][:, :],
                                     start=(i == 0), stop=(i == CT - 1))
                gq = tiny.tile([GROUPS, 8], F32, tag="gq", name=f"gq{b}")
                nc.vector.tensor_scalar_mul(gq[:, 0:1], pg[0:GROUPS, 0:1], INV_N)
                nc.vector.tensor_scalar_mul(gq[:, 1:2], pg[0:GROUPS, 1:2], INV_N)
                nc.vector.tensor_mul(gq[:, 2:3], gq[:, 0:1], gq[:, 0:1])
                nc.vector.tensor_sub(gq[:, 3:4], gq[:, 1:2], gq[:, 2:3])
                # rstd = rsqrt(var+eps) on DVE: magic-constant seed + 2 Newton
                # steps (keeps ScalarE's activation table pinned to Exp/Copy)
                vr = tiny.tile([GROUPS, 6], F32, tag="vr", name=f"vr{b}")
                vi = vr[:].bitcast(mybir.dt.int32)
                nc.vector.tensor_scalar_add(vr[:, 0:1], gq[:, 3:4], EPS)
                nc.vector.tensor_scalar(vi[:, 1:2], vi[:, 0:1], 1, None,
                                        op0=ALU.logical_shift_right)
                nc.vector.tensor_scalar(vi[:, 2:3], vi[:, 1:2], -1, 0x5f3759df,
                                        op0=ALU.mult, op1=ALU.add)
                for it in (3, 4):
                    nc.vector.tensor_mul(vr[:, 5:6], vr[:, 0:1],
                                         vr[:, it - 1:it])
                    nc.vector.tensor_mul(vr[:, 5:6], vr[:, 5:6],
                                         vr[:, it - 1:it])
                    nc.vector.tensor_scalar(vr[:, 5:6], vr[:, 5:6], -0.5, 1.5,
                                            op0=ALU.mult, op1=ALU.add)
                    nc.vector.tensor_mul(vr[:, it:it + 1], vr[:, it - 1:it],
                                         vr[:, 5:6])
                nc.vector.tensor_copy(gq[:, 5:6], vr[:, 4:5])
                nc.vector.tensor_mul(gq[:, 6:7], gq[:, 0:1], gq[:, 5:6])
                for i in range(CT):
                    pf = pmm.tile([128, 1024], F32, tag="mm", name=f"pf{b}_{i}")
                    nc.tensor.matmul(pf[0:128, 0:2], fan[i][:, :], gq[:, 5:7],
                                     start=True, stop=True)
                    scb = tiny.tile([128, 3], F32, tag="scb", name=f"scb{b}_{i}")
                    nc.vector.tensor_mul(scb[:, 0:1], pf[0:128, 0:1], nw[i][:])
                    nc.vector.tensor_mul(scb[:, 1:2], pf[0:128, 1:2], nw[i][:])
                    nc.vector.tensor_sub(scb[:, 2:3], nb[i][:], scb[:, 1:2])
                    t = xn_p.tile([128, L], MM_DT, tag="xn", name=f"xn{b}_{i}")
                    nc.vector.tensor_scalar(t[:], xs[(b, i)][:], scb[:, 0:1],
                                            scb[:, 2:3], op0=ALU.mult, op1=ALU.add)
                    xn[(b, i)] = t
                    # fold proj bias into the residual source (in place)
                    nc.vector.tensor_scalar_add(xs[(b, i)][:], xs[(b, i)][:],
                                                pb_col[i][:])

            def qkv_j(b, j):
                t = qk_p.tile([128, L], MM_DT, tag="qk", name=f"qk{b}_{j}")
                pq = pmm.tile([128, 1024], F32, tag="mm", name=f"pq{b}_{j}")
                for i in range(CT):
                    for n in range(2):
                        nsl = slice(512 * n, 512 * (n + 1))
                        nc.tensor.matmul(pq[:, nsl],
                                         wq[i][:, 128 * j:128 * (j + 1)],
                                         xn[(b, i)][:, nsl],
                                         start=(i == 0), stop=(i == CT - 1))
                nc.scalar.activation(t[:, :], pq[:, :], ACT.Identity,
                                     bias=qb_qk[j][:])
                qk[(b, j)] = t

            def vt_m(b, m):
                pvf = pmm.tile([128, 1024], F32, tag="mm", name=f"pv{b}_{m}")
                pv = pvf[:, 0:512]
                for i in range(CT):
                    nc.tensor.matmul(pv,
                                     xn[(b, i)][:, 128 * m:128 * (m + 1)],
                                     wq[i][:, 2 * C:3 * C],
                                     start=(i == 0), stop=False)
                nc.tensor.matmul(pv, ones_t[0:1, 0:128], qbv_t[0:1, :],
                                 start=False, stop=True)
                t = vt_p.tile([128, NH * (CH + 1)], MM_DT, tag="vt",
                              name=f"vt{b}_{m}")
                dst = t[:].rearrange("p (h x) -> p h x", x=CH + 1)
                nc.vector.tensor_copy(dst[:, :, 0:CH],
                                      pv.rearrange("p (h x) -> p h x", x=CH))
                nc.vector.memset(dst[:, :, CH:CH + 1], 1.0)
                vt[(b, m)] = t

            deferred = []   # normalize-emission units of the previous hp

            def defer_step(force=False):
                if deferred:
                    deferred.pop(0)()

            def attn_hp(b, hp, fctx=None):

                def fill_step(force=False):
                    if fctx is None or not fctx["fills"]:
                        return
                    if force:
                        fctx["fills"].pop(0)()
                        return
                    k = fctx["step"]
                    fctx["step"] += 1
                    if (k + 1) * fctx["U"] // fctx["S"] > k * fctx["U"] // fctx["S"]:
                        fctx["fills"].pop(0)()

                q_t = qk[(b, hp)]
                k_t = qk[(b, CT + hp)]
                ews = {}
                # ---- pass 1: scores + exp (full t) + av over t-half 0 ----
                pa_h = [pa_p.tile([CH + 1, 512], F32, tag="pa",
                                  name=f"pa0_{b}_{hp}_{hh}") for hh in range(2)]
                for m in range(ST):
                    defer_step()
                    msl = slice(128 * m, 128 * (m + 1))
                    pw = []
                    for hh, (plo, phi, tp) in enumerate(
                            ((0, CH, (0, 0)), (CH, 128, (CH, 0)))):
                        p = pmm.tile([128, 1024], F32, tag="mm",
                                     name=f"ps{b}_{hp}_{m}_{hh}")
                        for n in range(2):
                            nsl = slice(512 * n, 512 * (n + 1))
                            nc.tensor.matmul(p[:, nsl], k_t[plo:phi, msl],
                                             q_t[plo:phi, nsl],
                                             start=True, stop=True,
                                             tile_position=tp)
                        pw.append((hh, p))
                    for hh, p in pw:
                        if (hh + m) % 2 == 0:
                            e = ew_p.tile([128, L], I16, tag="ewi")
                            nc.vector.tensor_scalar(e[:], p[:, :], EXP_A, EXP_B,
                                                    op0=ALU.mult, op1=ALU.add)
                            ews[(m, hh)] = e.bitcast(MM_DT)
                        else:
                            e = ew_p.tile([128, L], MM_DT, tag="ewa")
                            nc.scalar.activation(e[:], p[:, :], ACT.Exp,
                                                 scale=SCALE2)
                            ews[(m, hh)] = e[:]
                    if m > 0:
                        for hh in range(2):
                            lhs = vt[(b, m - 1)][:, (2 * hp + hh) * (CH + 1):
                                                 (2 * hp + hh + 1) * (CH + 1)]
                            nc.tensor.matmul(pa_h[hh][:, :], lhs,
                                             ews[(m - 1, hh)][:, 0:512],
                                             start=(m - 1 == 0), stop=False)
                    fill_step()
                for hh in range(2):
                    lhs = vt[(b, ST - 1)][:, (2 * hp + hh) * (CH + 1):
                                          (2 * hp + hh + 1) * (CH + 1)]
                    nc.tensor.matmul(pa_h[hh][:, :], lhs,
                                     ews[(ST - 1, hh)][:, 0:512],
                                     start=False, stop=True)
                # stage t-half 0 out of PSUM (frees the banks for pass 2)
                stg = {}
                for hh in range(2):
                    s_t = stg_p.tile([CH + 1, 512], F32, tag="stg",
                                     name=f"stg{b}_{hp}_{hh}_0")
                    nc.vector.tensor_copy(s_t[:], pa_h[hh][:, :])
                    stg[(hh, 0)] = s_t
                zp0 = z_p.tile([8, 128], F32, tag="zp", name=f"zp0_{b}_{hp}")
                for hh in range(2):
                    nc.sync.dma_start(zp0[4 * hh:4 * hh + 4, :],
                                      stg[(hh, 0)][CH:CH + 1, :])
                zrp0 = z_p.tile([8, 128], F32, tag="zrp", name=f"zrp0_{b}_{hp}")
                nc.vector.reciprocal_approx_fast(zrp0[:], zp0[:])
                fill_step(force=True)
                # ---- pass 2: av over t-half 1 (ew tiles already in SBUF) ----
                pa_h2 = [pa_p.tile([CH + 1, 512], F32, tag="pa",
                                   name=f"pa1_{b}_{hp}_{hh}") for hh in range(2)]
                for m in range(ST):
                    for hh in range(2):
                        lhs = vt[(b, m)][:, (2 * hp + hh) * (CH + 1):
                                         (2 * hp + hh + 1) * (CH + 1)]
                        nc.tensor.matmul(pa_h2[hh][:, :], lhs,
                                         ews[(m, hh)][:, 512:1024],
                                         start=(m == 0), stop=(m == ST - 1))

                # ---- normalize: DEFERRED into the next hp's emission so the
                # stage/recip chain never blocks the exp stream in the FIFOs
                a_t = a_p.tile([128, L], MM_DT, tag="a", name=f"a{b}_{hp}")

                def u1():
                    for hh in range(2):
                        s_t = stg_p.tile([CH + 1, 512], F32, tag="stg",
                                         name=f"stg{b}_{hp}_{hh}_1")
                        nc.vector.tensor_copy(s_t[:], pa_h2[hh][:, :])
                        stg[(hh, 1)] = s_t

                zc = {0: zrp0}

                def u2():
                    zp1 = z_p.tile([8, 128], F32, tag="zp",
                                   name=f"zp1_{b}_{hp}")
                    for hh in range(2):
                        nc.sync.dma_start(zp1[4 * hh:4 * hh + 4, :],
                                          stg[(hh, 1)][CH:CH + 1, :])
                    zrp1 = z_p.tile([8, 128], F32, tag="zrp",
                                    name=f"zrp1_{b}_{hp}")
                    nc.vector.reciprocal_approx_fast(zrp1[:], zp1[:])
                    zc[1] = zrp1

                def mk_u34(hh):
                    def u():
                        for n in range(2):
                            rzf = z_p.tile([1, 512], F32, tag="rzf",
                                           name=f"rzf{b}_{hp}_{hh}_{n}")
                            nc.sync.dma_start(rzf[0:1, :],
                                              zc[n][4 * hh:4 * hh + 4, :])
                            rzb = zb_p.tile([CH, 512], F32, tag="zb")
                            nc.gpsimd.partition_broadcast(rzb[:], rzf[:])
                            rows = slice(CH * hh, CH * (hh + 1))
                            nsl = slice(512 * n, 512 * (n + 1))
                            nc.vector.tensor_mul(a_t[rows, nsl],
                                                 stg[(hh, n)][0:CH, :],
                                                 rzb[:, :])
                    return u

                deferred.extend([u1, u2, mk_u34(0), mk_u34(1)])
                a_tiles[(b, hp)] = a_t

            def proj_j(b, j):
                o_t = out_p.tile([128, L], F32, tag="o", name=f"o{b}_{j}")
                pp = pmm.tile([128, 1024], F32, tag="mm", name=f"pp{b}_{j}")
                for i in range(CT):
                    for n in range(2):
                        nsl = slice(512 * n, 512 * (n + 1))
                        nc.tensor.matmul(pp[:, nsl],
                                         wp[i][:, 128 * j:128 * (j + 1)],
                                         a_tiles[(b, i)][:, nsl],
                                         start=(i == 0), stop=(i == CT - 1))
                nc.vector.tensor_add(o_t[:, :], pp[:, :], xs[(b, j)][:, :])
                nc.sync.dma_start(out_d[b, 128 * j:128 * (j + 1), :], o_t[:])

            # ---------------- schedule ----------------
            # PE warm-up: real-shaped dummy matmuls while DMA/stats run, so
            # the HAM un-throttles before qkv(0) begins
            def warm(n):
                wu = pmm.tile([128, 1024], F32, tag="mm",
                              name=f"warm{nc.next_id()}")
                for _ in range(n):
                    nc.tensor.matmul(wu[:, 0:512], warm_t[:, 0:128],
                                     warm_t[:, 0:512], start=True, stop=True)

            warm(44)
            gn_pre(0)
            gn_post(0)
            for j in range(2 * CT):
                qkv_j(0, j)
                if j == 0:
                    gn_pre(1)      # sample-1 stats on DVE under qkv(0) PE work
            for m in range(ST):
                vt_m(0, m)
            gn_post(1)
            # attention(0) interleaved per-m with qkv/vT of sample 1
            fills1 = ([(lambda jj: (lambda: qkv_j(1, jj)))(j) for j in range(2 * CT)]
                      + [(lambda mm_: (lambda: vt_m(1, mm_)))(m) for m in range(ST)])
            fctx1 = {"fills": fills1, "step": 0, "U": len(fills1), "S": 32}
            for hp in range(NH // 2):
                attn_hp(0, hp, fctx1)
            while fctx1["fills"]:
                fctx1["fills"].pop(0)()
            # attention(1) interleaved per-m with proj of sample 0
            fills2 = [(lambda jj: (lambda: proj_j(0, jj)))(j) for j in range(CT)]
            fctx2 = {"fills": fills2, "step": 0, "U": len(fills2), "S": 34}
            for hp in range(NH // 2):
                attn_hp(1, hp, fctx2)
            while deferred:
                deferred.pop(0)()
            while fctx2["fills"]:
                fctx2["fills"].pop(0)()
            for j in range(CT):
                proj_j(1, j)

    nc.compile()
    return nc


_prog_cache = {}


def _get_program():
    if "p" not in _prog_cache:
        _prog_cache["p"] = _build_program()
    return _prog_cache["p"]


def _host_constants():
    sel = np.zeros((CT, 128, GROUPS), dtype=np.float32)
    fan = np.zeros((CT, GROUPS, 128), dtype=np.float32)
    for i in range(CT):
        for p in range(128):
            g = (128 * i + p) // GS
            sel[i, p, g] = 1.0
            fan[i, g, p] = 1.0
    ones = np.ones((1, 512), dtype=mybir.dt.np(MM_DT))
    return sel, fan, ones


def kernel(x, norm_w, norm_b, qkv_w, qkv_b, proj_w, proj_b):
    global LAST_RESULTS
    x = np.ascontiguousarray(np.asarray(x, dtype=np.float32).astype(mybir.dt.np(MM_DT)))
    np_mm = mybir.dt.np(MM_DT)
    wqkvT = np.ascontiguousarray(np.asarray(qkv_w, dtype=np.float32).T.astype(np_mm))
    wprojT = np.ascontiguousarray(np.asarray(proj_w, dtype=np.float32).T.astype(np_mm))
    sel, fan, ones = _host_constants()

    xr = x.reshape(B, C, L)
    nc = _get_program()

    common = {
        "wqkvT": wqkvT,
        "wprojT": wprojT,
        "norm_w": np.ascontiguousarray(norm_w, dtype=np.float32),
        "norm_b": np.ascontiguousarray(norm_b, dtype=np.float32),
        "qkv_b": np.ascontiguousarray(qkv_b, dtype=np.float32),
        "proj_b": np.ascontiguousarray(proj_b, dtype=np.float32),
        "sel": sel,
        "fan": fan,
        "ones": ones,
    }
    in_maps = []
    for c in range(N_CORES):
        m = dict(common)
        m["x"] = np.ascontiguousarray(xr[BL * c:BL * (c + 1)])
        in_maps.append(m)

    trace = os.environ.get("KERNEL_TRACE", "0") == "1"
    kwargs = {}
    if trace:
        kwargs = dict(trace=True, trace_cores=[0])
    res = run_bass_kernel_spmd(nc, in_maps, core_ids=list(range(N_CORES)),
                               **kwargs)
    LAST_RESULTS = res
    out = np.concatenate([res.results[c]["out"] for c in range(N_CORES)], axis=0)
    return out.reshape(B, C, HH, WW)
